# revision 25
# baseline (speedup 1.0000x reference)
"""Trainium2 Bass kernel for nn_CrossAttention (B=2, N=2048, C=1024, H=16).

Sharding: 16 heads / 8 cores = 2 heads per core (both batches on every
core).  Each core computes its heads' Q/K/V projections with the matching
128-row slice of Wq/Wk/Wv, full attention for its 4 (batch, head) pairs,
and a partial output projection against its 128-column slice of Wp.  The
host sums the 8 partial projections (the tensor-parallel all-reduce) and
adds the bias.

Schedule (per core): the exp of 16.8M score elements on the Scalar engine
(~135us at [128,1024] grain) is the pacing resource.  Work is organized
as four scores units (batch x query-half) streaming through ScalarE,
with attnv/denominator matmuls for the previous unit, Q/K/V projection
quarters, PE v-transposes, and output-projection pieces woven into each
unit's PE slack.  All large SBUF tensors are split per (batch, half) so
Tile's per-tile dependency tracking doesn't serialize across units.

PSUM budget (8 banks of 2KB/partition):
  sc pool   2 x [128,1024] f32 = 4 banks  (score regions / S0 qkv / proj)
  o  pool   1 x [128,1024] f32 = 2 banks  (attnv accum / woven qkv)
  d  pool   1 x [128,1024] f32 = 2 banks  (denominators / v-transposes)

On-device layouts (per core, fp16 matmul operands / fp32 PSUM):
  xq/xt    [1024 ch, 4096 pos]  channel-major inputs (host pre-transposed)
  q2T/k2T/v2T  4 x [128, 1024]  per (batch, half); rows 0-63 head0 dims
  vpos     2 x [128, 2048]      per batch, v transposed key-major via PE
  S_T      [128 keys, 1024 q]   exp(scores^T) tile per (b, h, m, qp), fp16
  outT     4 x [128, 1024]      normalized attention output per (b, qp)
  out_p    [2, 2048, 1024] f32  partial projection (summed on host)
"""

import os
import sys

for _p in ("/opt/trn_rl_repo", os.path.expanduser("~/.axon_site/_ro/trn_rl_repo")):
    if os.path.isdir(_p) and _p not in sys.path:
        sys.path.insert(0, _p)

import numpy as np

import concourse.bacc as bacc
import concourse.mybir as mybir
import concourse.tile as tile
from concourse.bass_utils import run_bass_kernel_spmd

F16 = mybir.dt.float16
F32 = mybir.dt.float32
AF = mybir.ActivationFunctionType

B, N, C, H, D = 2, 2048, 1024, 16, 64
NCORES = 8
SCALE = float(D) ** -0.5

TRACE = False
LAST_EXEC_NS = None
LAST_RESULTS = None

_COMPILED_NC = None


def _emit(nc):
    xq = nc.dram_tensor("xq", [C, B * N], F16, kind="ExternalInput")
    xt = nc.dram_tensor("xt", [C, B * N], F16, kind="ExternalInput")
    wq = nc.dram_tensor("wq", [128, 1024], F16, kind="ExternalInput")
    wk = nc.dram_tensor("wk", [128, 1024], F16, kind="ExternalInput")
    wv = nc.dram_tensor("wv", [128, 1024], F16, kind="ExternalInput")
    wp = nc.dram_tensor("wp", [128, C], F16, kind="ExternalInput")
    id64 = nc.dram_tensor("id64", [128, 64], F16, kind="ExternalInput")
    outp = nc.dram_tensor("out_p", [B, N, C], F32, kind="ExternalOutput")

    with tile.TileContext(nc) as tc:
        with (
            tc.tile_pool(name="consts", bufs=1) as cpool,
            tc.tile_pool(name="xs", bufs=6) as xs,
            tc.tile_pool(name="big", bufs=1) as big,
            tc.tile_pool(name="stp", bufs=32) as stp,
            tc.tile_pool(name="ob", bufs=1) as obp,
            tc.tile_pool(name="rc", bufs=2) as rcpool,
            tc.tile_pool(name="pe", bufs=2) as pep,
            tc.tile_pool(name="sc", bufs=1, space="PSUM") as scp,
            tc.tile_pool(name="o", bufs=1, space="PSUM") as op,
            tc.tile_pool(name="d", bufs=1, space="PSUM") as dp,
        ):
            # ---- constants ------------------------------------------------
            w_sb = {}
            for name, dram in (("wq", wq), ("wk", wk), ("wv", wv)):
                t_ = cpool.tile([128, 1024], F16, tag=name, name=f"w_{name}")
                nc.sync.dma_start(t_[:], dram[:])
                w_sb[name] = t_
            wp_sb = cpool.tile([128, C], F16, tag="wp")
            id_sb = cpool.tile([128, 64], F16, tag="id64")
            ones_sb = cpool.tile([128, 1], F16, tag="ones")
            nc.vector.memset(ones_sb[:], 1.0)

            def late_consts():
                nc.sync.dma_start(id_sb[:], id64[:])
                nc.sync.dma_start(wp_sb[:], wp[:])

            # per-(batch, half) activation tiles, [128, 1024] fp16 each
            q2T = {}
            k2T = {}
            v2T = {}
            vpos = {}
            outT = {}
            for b in range(2):
                vpos[b] = big.tile([128, 2048], F16, tag=f"vpos{b}", name=f"vpos{b}")
                for hf in range(2):
                    q2T[(b, hf)] = big.tile([128, 1024], F16, tag=f"q{b}{hf}", name=f"q2T{b}{hf}")
                    k2T[(b, hf)] = big.tile([128, 1024], F16, tag=f"k{b}{hf}", name=f"k2T{b}{hf}")
                    v2T[(b, hf)] = big.tile([128, 1024], F16, tag=f"v{b}{hf}", name=f"v2T{b}{hf}")
                    outT[(b, hf)] = obp.tile(
                        [128, 1024], F16, tag=f"outT{b}{hf}", name=f"outT{b}{hf}"
                    )

            st = {}    # (b, h, m, qp) -> S_T tile
            ps_o = {}  # (b, qp) -> attnv accumulator
            ps_d = {}  # b -> denominator psum

            # ---- emission helpers -----------------------------------------
            def q_quarter(b, hf, pool, ptag):
                cols = slice(b * 2048 + hf * 1024, b * 2048 + (hf + 1) * 1024)
                ps_q = pool.tile([128, 1024], F32, tag=ptag, name=f"psq{b}{hf}")
                for kc in range(8):
                    x_t = xs.tile([128, 1024], F16, tag="x", name=f"xq{b}{hf}{kc}")
                    nc.sync.dma_start(x_t[:], xq[kc * 128 : (kc + 1) * 128, cols])
                    for qc in range(2):
                        cs = slice(qc * 512, (qc + 1) * 512)
                        nc.tensor.matmul(
                            ps_q[:, cs],
                            lhsT=w_sb["wq"][:, kc * 128 : (kc + 1) * 128],
                            rhs=x_t[:, cs],
                            start=(kc == 0),
                            stop=(kc == 7),
                        )
                    if kc % 2 == 1:
                        yield
                nc.vector.tensor_copy(q2T[(b, hf)][:], ps_q[:])

            def kv_quarter(b, hf, poolk, ktag, poolv, vtag):
                cols = slice(b * 2048 + hf * 1024, b * 2048 + (hf + 1) * 1024)
                ps_k = poolk.tile([128, 1024], F32, tag=ktag, name=f"psk{b}{hf}")
                ps_v = poolv.tile([128, 1024], F32, tag=vtag, name=f"psv{b}{hf}")
                for kc in range(8):
                    x_t = xs.tile([128, 1024], F16, tag="x", name=f"xt{b}{hf}{kc}")
                    nc.sync.dma_start(x_t[:], xt[kc * 128 : (kc + 1) * 128, cols])
                    for qc in range(2):
                        cs = slice(qc * 512, (qc + 1) * 512)
                        nc.tensor.matmul(
                            ps_k[:, cs],
                            lhsT=w_sb["wk"][:, kc * 128 : (kc + 1) * 128],
                            rhs=x_t[:, cs],
                            start=(kc == 0),
                            stop=(kc == 7),
                        )
                        nc.tensor.matmul(
                            ps_v[:, cs],
                            lhsT=w_sb["wv"][:, kc * 128 : (kc + 1) * 128],
                            rhs=x_t[:, cs],
                            start=(kc == 0),
                            stop=(kc == 7),
                        )
                    if kc % 2 == 1:
                        yield
                nc.vector.tensor_copy(k2T[(b, hf)][:], ps_k[:])
                nc.vector.tensor_copy(v2T[(b, hf)][:], ps_v[:])

            def transposes(b):
                for h in range(2):
                    hp = slice(h * 64, (h + 1) * 64)
                    for oct_ in range(2):
                        ps_t = dp.tile(
                            [128, 512], F16, tag="d", name=f"pst{b}{h}{oct_}"
                        )
                        for i in range(8):
                            m = oct_ * 8 + i
                            src = v2T[(b, m // 8)]
                            ks = slice((m % 8) * 128, (m % 8 + 1) * 128)
                            nc.tensor.transpose(
                                ps_t[:, i * 64 : (i + 1) * 64],
                                src[hp, ks],
                                id_sb[hp, :],
                            )
                        nc.vector.tensor_copy(
                            vpos[b][
                                :,
                                h * 1024 + oct_ * 512 : h * 1024 + (oct_ + 1) * 512,
                            ],
                            ps_t[:],
                        )
                        yield

            def scores(b, m, qp):
                # one [128, 2048] PSUM region holds both heads' scores for
                # (m, qp); a single exp instruction drains it (larger grain
                # amortizes the ~230-cycle ACT per-instruction overhead)
                ps = scp.tile([128, 2048], F32, tag="sc", name=f"sc{b}{m}{qp}")
                kt = k2T[(b, m // 8)]
                ms = slice((m % 8) * 128, (m % 8 + 1) * 128)
                for qc in range(2):
                    cs = slice(qc * 512, (qc + 1) * 512)
                    for h in range(2):
                        hp = slice(h * 64, (h + 1) * 64)
                        nc.tensor.matmul(
                            ps[:, h * 1024 + qc * 512 : h * 1024 + (qc + 1) * 512],
                            lhsT=kt[hp, ms],
                            rhs=q2T[(b, qp)][hp, cs],
                            start=True,
                            stop=True,
                        )
                s = stp.tile([128, 2048], F16, tag="st", name=f"st{b}{m}{qp}")
                nc.scalar.activation(s[:], ps[:], AF.Exp, scale=SCALE)
                st[(b, m, qp)] = s

            def attnv(b, qp, m):
                if m == 0:
                    ps_o[(b, qp)] = op.tile(
                        [128, 1024], F32, tag="o", name=f"o{b}{qp}"
                    )
                    if qp == 0:
                        ps_d[b] = dp.tile([128, 1024], F32, tag="d", name=f"d{b}")
                po = ps_o[(b, qp)]
                pd = ps_d[b]
                kw = dict(start=(m == 0), stop=(m == 15))
                stile = st[(b, m, qp)]
                for qc in range(2):
                    cs = slice(qc * 512, (qc + 1) * 512)
                    for h in range(2):
                        nc.tensor.matmul(
                            po[h * 64 : (h + 1) * 64, cs],
                            lhsT=vpos[b][:, h * 1024 + m * 64 : h * 1024 + (m + 1) * 64],
                            rhs=stile[:, h * 1024 + qc * 512 : h * 1024 + (qc + 1) * 512],
                            **kw,
                        )
                ds = slice(qp * 512, (qp + 1) * 512)
                for qc in range(2):
                    for h in range(2):
                        row = h * 32 + qc * 64
                        nc.tensor.matmul(
                            pd[row : row + 1, ds],
                            lhsT=ones_sb[:, 0:1],
                            rhs=stile[:, h * 1024 + qc * 512 : h * 1024 + (qc + 1) * 512],
                            skip_group_check=True,
                            tile_position=(0, row),
                            **kw,
                        )

            def normalize(b, qp):
                # ps_o eviction first (frees the o slot the next attnv pass
                # blocks on), then one approx-reciprocal over every denom
                # row at once (frees d), then broadcast + multiply from
                # SBUF off the critical path.
                po = ps_o[(b, qp)]
                pd = ps_d[b]
                ds = slice(qp * 512, (qp + 1) * 512)
                rc = rcpool.tile([128, 1024], F32, tag="rc", name=f"rc{b}{qp}")
                ev_o = pep.tile([128, 1024], F32, tag="pe", name=f"evo{b}{qp}")
                bcast = [0] * 32
                nc.vector.tensor_copy(ev_o[:], po[:])
                # denom rows (h,qc) -> h*32 + qc*64; ~18-bit reciprocal is
                # ample for a softmax denominator; garbage lanes are never
                # selected by the broadcast below.
                nc.vector.reciprocal_approx_fast(rc[:, 0:512], pd[:, ds])
                c0 = slice(0, 512)
                c1 = slice(512, 1024)
                # qc1 dests (cols 512+) first: sources rows 64/96 cols 0:512
                for dst, srow in ((0, 64), (32, 64), (64, 96), (96, 96)):
                    nc.vector.stream_shuffle(
                        rc[dst : dst + 32, c1], rc[srow : srow + 32, c0], bcast
                    )
                # then qc0 dests in overwrite-safe order
                for dst, srow in ((64, 32), (96, 32), (32, 0), (0, 0)):
                    nc.vector.stream_shuffle(
                        rc[dst : dst + 32, c0], rc[srow : srow + 32, c0], bcast
                    )
                nc.vector.tensor_mul(outT[(b, qp)][:], ev_o[:], rc[:])

            def proj_piece(b, qp, pm, use_act=False):
                # one 128-position tile of the output projection
                ps_p = scp.tile([128, 1024], F32, tag="sc", name=f"pp{b}{qp}{pm}")
                for ncol in range(2):
                    nc.tensor.matmul(
                        ps_p[:, ncol * 512 : (ncol + 1) * 512],
                        lhsT=outT[(b, qp)][:, pm * 128 : (pm + 1) * 128],
                        rhs=wp_sb[:, ncol * 512 : (ncol + 1) * 512],
                        start=True,
                        stop=True,
                    )
                ev = pep.tile([128, 1024], F32, tag="pe", name=f"pe{b}{qp}{pm}")
                if use_act:
                    # tail only: ScalarE is idle, split the eviction
                    nc.vector.tensor_copy(ev[:, 0:512], ps_p[:, 0:512])
                    nc.scalar.copy(ev[:, 512:1024], ps_p[:, 512:1024])
                else:
                    # mid-stream: keep ScalarE exclusively on exp
                    nc.vector.tensor_copy(ev[:], ps_p[:])
                rows = slice(qp * 1024 + pm * 128, qp * 1024 + (pm + 1) * 128)
                nc.sync.dma_start(outp[b, rows, :], ev[:])

            def drain(gen):
                for _ in gen:
                    pass

            # ---- S0: minimal prefix — what scores(b0, m<8, qp0) needs -----
            drain(q_quarter(0, 0, scp, "sc"))
            drain(kv_quarter(0, 0, scp, "sc", op, "o"))

            late_consts()

            # ---- U0a (m 0-7): scores(b0,qp0) | rest of b0 qkv -------------
            def u0a_gen():
                yield from q_quarter(0, 1, op, "o")
                yield from kv_quarter(0, 1, op, "o", dp, "d")

            gen = u0a_gen()
            for m in range(8):
                scores(0, m, 0)
                next(gen, None)
            drain(gen)

            # ---- U0b (m 8-15): scores(b0,qp0) | b1 qkv + both transposes --
            def u0b_gen():
                yield from transposes(0)
                yield from q_quarter(1, 0, op, "o")
                yield from q_quarter(1, 1, op, "o")
                yield from kv_quarter(1, 0, op, "o", dp, "d")
                yield from kv_quarter(1, 1, op, "o", dp, "d")
                yield from transposes(1)

            gen = u0b_gen()
            for m in range(8, 16):
                scores(0, m, 0)
                next(gen, None)
                next(gen, None)
                next(gen, None)
            drain(gen)

            # ---- U1: scores(b0,qp1) | attnv(b0,qp0) -----------------------
            for m in range(16):
                scores(0, m, 1)
                attnv(0, 0, m)
            normalize(0, 0)
            # ---- U2: scores(b1,qp0) | attnv(b0,qp1) | proj(b0,qp0) --------
            for m in range(16):
                scores(1, m, 0)
                attnv(0, 1, m)
                if m % 2 == 1:
                    proj_piece(0, 0, m // 2)
            normalize(0, 1)
            # ---- U3: scores(b1,qp1) | attnv(b1,qp0) | proj(b0,qp1) --------
            for m in range(16):
                scores(1, m, 1)
                attnv(1, 0, m)
                if m % 2 == 1:
                    proj_piece(0, 1, m // 2)
            normalize(1, 0)
            # ---- U4: attnv(b1,qp1) | proj(b1,qp0) -------------------------
            for m in range(16):
                attnv(1, 1, m)
                if m % 2 == 1:
                    proj_piece(1, 0, m // 2, use_act=True)
            normalize(1, 1)
            for pm in range(8):
                proj_piece(1, 1, pm, use_act=True)
    return nc


def _get_compiled():
    global _COMPILED_NC
    if _COMPILED_NC is None:
        nc = bacc.Bacc(
            "TRN2", target_bir_lowering=False, debug=False, num_devices=NCORES
        )
        _emit(nc)
        nc.compile()
        _COMPILED_NC = nc
    return _COMPILED_NC


def _install_trace_shim():
    """Register antenv.axon_hooks NTFF hook (missing on this image)."""
    import contextlib
    import ctypes
    import types

    if "antenv.axon_hooks" in sys.modules:
        return
    try:
        import antenv
    except ImportError:
        return
    so_path = "/opt/axon/libaxon_pjrt.so"
    if not os.path.exists(so_path):
        return

    mod = types.ModuleType("antenv.axon_hooks")
    mod._hook = None
    mod.set_axon_ntff_profile_hook = lambda h: setattr(mod, "_hook", h)
    mod.get_axon_ntff_profile_hook = lambda: mod._hook

    lib = ctypes.CDLL(so_path)
    if not hasattr(lib, "axon_start_nrt_profile"):
        return
    lib.axon_start_nrt_profile.argtypes = [
        ctypes.POINTER(ctypes.c_int64),
        ctypes.c_size_t,
    ]
    lib.axon_start_nrt_profile.restype = ctypes.c_int64
    lib.axon_stop_nrt_profile.argtypes = [ctypes.c_char_p]
    lib.axon_stop_nrt_profile.restype = ctypes.c_int64

    @contextlib.contextmanager
    def _hook(output_dir, device_ids):
        import jax

        jax.devices()
        if device_ids:
            ids = (ctypes.c_int64 * len(device_ids))(*device_ids)
            rc = lib.axon_start_nrt_profile(ids, len(device_ids))
        else:
            rc = lib.axon_start_nrt_profile(None, 0)
        if rc != 0:
            raise RuntimeError(f"axon_start_nrt_profile rc={rc}")
        try:
            yield
        finally:
            n = lib.axon_stop_nrt_profile(str(output_dir).encode())
            if n < 0:
                raise RuntimeError(f"axon_stop_nrt_profile rc={n}")

    mod.set_axon_ntff_profile_hook(_hook)
    sys.modules["antenv.axon_hooks"] = mod
    antenv.axon_hooks = mod


def kernel(query, target, Wq, Wk, Wv, Wp, bp):
    global LAST_EXEC_NS, LAST_RESULTS
    query = np.asarray(query, dtype=np.float32)
    target = np.asarray(target, dtype=np.float32)
    Wq = np.asarray(Wq, dtype=np.float32)
    Wk = np.asarray(Wk, dtype=np.float32)
    Wv = np.asarray(Wv, dtype=np.float32)
    Wp = np.asarray(Wp, dtype=np.float32)
    bp = np.asarray(bp, dtype=np.float32)

    xq = np.ascontiguousarray(query.reshape(B * N, C).T).astype(np.float16)
    xt = np.ascontiguousarray(target.reshape(B * N, C).T).astype(np.float16)
    id64 = np.zeros((128, 64), dtype=np.float16)
    for p in range(128):
        id64[p, p % 64] = 1.0

    def wlayout(Wm, rows):
        # SBUF weight tile [p, kc*128 + m] = W[row0 + m, kc*128 + p]
        ws = Wm[rows, :].astype(np.float16)  # (128, 1024)
        return np.ascontiguousarray(
            ws.reshape(128, 8, 128).transpose(2, 1, 0).reshape(128, 1024)
        )

    in_maps = []
    for c in range(NCORES):
        rows = slice(c * 128, (c + 1) * 128)
        in_maps.append(
            {
                "xq": xq,
                "xt": xt,
                "wq": wlayout(Wq, rows),
                "wk": wlayout(Wk, rows),
                "wv": wlayout(Wv, rows),
                "wp": np.ascontiguousarray(Wp[:, rows].T).astype(np.float16),
                "id64": id64,
            }
        )

    if TRACE:
        _install_trace_shim()

    nc = _get_compiled()
    res = run_bass_kernel_spmd(
        nc, in_maps, core_ids=list(range(NCORES)), trace=TRACE
    )
    LAST_RESULTS = res
    LAST_EXEC_NS = res.exec_time_ns

    acc = res.results[0]["out_p"].astype(np.float64)
    for c in range(1, NCORES):
        acc += res.results[c]["out_p"]
    out = acc.astype(np.float32) + bp[None, None, :]
    return out


# revision 26
# speedup vs baseline: 1.0219x; 1.0219x over previous
"""Trainium2 Bass kernel for nn_CrossAttention (B=2, N=2048, C=1024, H=16).

Sharding: 16 heads / 8 cores = 2 heads per core (both batches on every
core).  Each core computes its heads' Q/K/V projections with the matching
128-row slice of Wq/Wk/Wv, full attention for its 4 (batch, head) pairs,
and a partial output projection against its 128-column slice of Wp.  The
host sums the 8 partial projections (the tensor-parallel all-reduce) and
adds the bias.

Schedule (per core): the exp of 16.8M score elements on the Scalar engine
(~135us at [128,1024] grain) is the pacing resource.  Work is organized
as four scores units (batch x query-half) streaming through ScalarE,
with attnv/denominator matmuls for the previous unit, Q/K/V projection
quarters, PE v-transposes, and output-projection pieces woven into each
unit's PE slack.  All large SBUF tensors are split per (batch, half) so
Tile's per-tile dependency tracking doesn't serialize across units.

PSUM budget (8 banks of 2KB/partition):
  sc pool   2 x [128,1024] f32 = 4 banks  (score regions / S0 qkv / proj)
  o  pool   1 x [128,1024] f32 = 2 banks  (attnv accum / woven qkv)
  d  pool   1 x [128,1024] f32 = 2 banks  (denominators / v-transposes)

On-device layouts (per core, fp16 matmul operands / fp32 PSUM):
  xq/xt    [1024 ch, 4096 pos]  channel-major inputs (host pre-transposed)
  q2T/k2T/v2T  4 x [128, 1024]  per (batch, half); rows 0-63 head0 dims
  vpos     2 x [128, 2048]      per batch, v transposed key-major via PE
  S_T      [128 keys, 1024 q]   exp(scores^T) tile per (b, h, m, qp), fp16
  outT     4 x [128, 1024]      normalized attention output per (b, qp)
  out_p    [2, 2048, 1024] f32  partial projection (summed on host)
"""

import os
import sys

for _p in ("/opt/trn_rl_repo", os.path.expanduser("~/.axon_site/_ro/trn_rl_repo")):
    if os.path.isdir(_p) and _p not in sys.path:
        sys.path.insert(0, _p)

import numpy as np

import concourse.bacc as bacc
import concourse.mybir as mybir
import concourse.tile as tile
from concourse.bass_utils import run_bass_kernel_spmd

F16 = mybir.dt.float16
F32 = mybir.dt.float32
AF = mybir.ActivationFunctionType

B, N, C, H, D = 2, 2048, 1024, 16, 64
NCORES = 8
SCALE = float(D) ** -0.5

TRACE = False
LAST_EXEC_NS = None
LAST_RESULTS = None

_COMPILED_NC = None


def _emit(nc):
    xq = nc.dram_tensor("xq", [C, B * N], F16, kind="ExternalInput")
    xt = nc.dram_tensor("xt", [C, B * N], F16, kind="ExternalInput")
    wq = nc.dram_tensor("wq", [128, 1024], F16, kind="ExternalInput")
    wk = nc.dram_tensor("wk", [128, 1024], F16, kind="ExternalInput")
    wv = nc.dram_tensor("wv", [128, 1024], F16, kind="ExternalInput")
    wp = nc.dram_tensor("wp", [128, C], F16, kind="ExternalInput")
    id64 = nc.dram_tensor("id64", [128, 64], F16, kind="ExternalInput")
    outp = nc.dram_tensor("out_p", [B, N, C], F32, kind="ExternalOutput")

    with tile.TileContext(nc) as tc:
        with (
            tc.tile_pool(name="consts", bufs=1) as cpool,
            tc.tile_pool(name="xs", bufs=6) as xs,
            tc.tile_pool(name="big", bufs=1) as big,
            tc.tile_pool(name="stp", bufs=40) as stp,
            tc.tile_pool(name="ob", bufs=1) as obp,
            tc.tile_pool(name="rc", bufs=2) as rcpool,
            tc.tile_pool(name="pe", bufs=2) as pep,
            tc.tile_pool(name="sc", bufs=2, space="PSUM") as scp,
            tc.tile_pool(name="o", bufs=1, space="PSUM") as op,
            tc.tile_pool(name="d", bufs=1, space="PSUM") as dp,
        ):
            # ---- constants ------------------------------------------------
            w_sb = {}
            for name, dram in (("wq", wq), ("wk", wk), ("wv", wv)):
                t_ = cpool.tile([128, 1024], F16, tag=name, name=f"w_{name}")
                nc.sync.dma_start(t_[:], dram[:])
                w_sb[name] = t_
            wp_sb = cpool.tile([128, C], F16, tag="wp")
            id_sb = cpool.tile([128, 64], F16, tag="id64")
            ones_sb = cpool.tile([128, 1], F16, tag="ones")
            nc.vector.memset(ones_sb[:], 1.0)

            def late_consts():
                nc.sync.dma_start(id_sb[:], id64[:])
                nc.sync.dma_start(wp_sb[:], wp[:])

            # per-(batch, half) activation tiles, [128, 1024] fp16 each
            q2T = {}
            k2T = {}
            v2T = {}
            vpos = {}
            outT = {}
            for b in range(2):
                vpos[b] = big.tile([128, 2048], F16, tag=f"vpos{b}", name=f"vpos{b}")
                for hf in range(2):
                    q2T[(b, hf)] = big.tile([128, 1024], F16, tag=f"q{b}{hf}", name=f"q2T{b}{hf}")
                    k2T[(b, hf)] = big.tile([128, 1024], F16, tag=f"k{b}{hf}", name=f"k2T{b}{hf}")
                    v2T[(b, hf)] = big.tile([128, 1024], F16, tag=f"v{b}{hf}", name=f"v2T{b}{hf}")
                    outT[(b, hf)] = obp.tile(
                        [128, 1024], F16, tag=f"outT{b}{hf}", name=f"outT{b}{hf}"
                    )

            st = {}    # (b, h, m, qp) -> S_T tile
            ps_o = {}  # (b, qp) -> attnv accumulator
            ps_d = {}  # b -> denominator psum

            # ---- emission helpers -----------------------------------------
            def q_quarter(b, hf, pool, ptag):
                cols = slice(b * 2048 + hf * 1024, b * 2048 + (hf + 1) * 1024)
                ps_q = pool.tile([128, 1024], F32, tag=ptag, name=f"psq{b}{hf}")
                for kc in range(8):
                    x_t = xs.tile([128, 1024], F16, tag="x", name=f"xq{b}{hf}{kc}")
                    nc.sync.dma_start(x_t[:], xq[kc * 128 : (kc + 1) * 128, cols])
                    for qc in range(2):
                        cs = slice(qc * 512, (qc + 1) * 512)
                        nc.tensor.matmul(
                            ps_q[:, cs],
                            lhsT=w_sb["wq"][:, kc * 128 : (kc + 1) * 128],
                            rhs=x_t[:, cs],
                            start=(kc == 0),
                            stop=(kc == 7),
                        )
                    if kc % 2 == 1:
                        yield
                nc.vector.tensor_copy(q2T[(b, hf)][:], ps_q[:])

            def kv_quarter(b, hf, poolk, ktag, poolv, vtag):
                cols = slice(b * 2048 + hf * 1024, b * 2048 + (hf + 1) * 1024)
                ps_k = poolk.tile([128, 1024], F32, tag=ktag, name=f"psk{b}{hf}")
                ps_v = poolv.tile([128, 1024], F32, tag=vtag, name=f"psv{b}{hf}")
                for kc in range(8):
                    x_t = xs.tile([128, 1024], F16, tag="x", name=f"xt{b}{hf}{kc}")
                    nc.sync.dma_start(x_t[:], xt[kc * 128 : (kc + 1) * 128, cols])
                    for qc in range(2):
                        cs = slice(qc * 512, (qc + 1) * 512)
                        nc.tensor.matmul(
                            ps_k[:, cs],
                            lhsT=w_sb["wk"][:, kc * 128 : (kc + 1) * 128],
                            rhs=x_t[:, cs],
                            start=(kc == 0),
                            stop=(kc == 7),
                        )
                        nc.tensor.matmul(
                            ps_v[:, cs],
                            lhsT=w_sb["wv"][:, kc * 128 : (kc + 1) * 128],
                            rhs=x_t[:, cs],
                            start=(kc == 0),
                            stop=(kc == 7),
                        )
                    if kc % 2 == 1:
                        yield
                nc.vector.tensor_copy(k2T[(b, hf)][:], ps_k[:])
                nc.vector.tensor_copy(v2T[(b, hf)][:], ps_v[:])

            def transposes(b):
                for h in range(2):
                    hp = slice(h * 64, (h + 1) * 64)
                    for oct_ in range(2):
                        ps_t = dp.tile(
                            [128, 512], F16, tag="d", name=f"pst{b}{h}{oct_}"
                        )
                        for i in range(8):
                            m = oct_ * 8 + i
                            src = v2T[(b, m // 8)]
                            ks = slice((m % 8) * 128, (m % 8 + 1) * 128)
                            nc.tensor.transpose(
                                ps_t[:, i * 64 : (i + 1) * 64],
                                src[hp, ks],
                                id_sb[hp, :],
                            )
                        nc.vector.tensor_copy(
                            vpos[b][
                                :,
                                h * 1024 + oct_ * 512 : h * 1024 + (oct_ + 1) * 512,
                            ],
                            ps_t[:],
                        )
                        yield

            def scores(b, m, qp):
                ps = [
                    scp.tile([128, 1024], F32, tag="sc", name=f"sc{b}{m}{qp}{h}")
                    for h in range(2)
                ]
                kt = k2T[(b, m // 8)]
                ms = slice((m % 8) * 128, (m % 8 + 1) * 128)
                for qc in range(2):
                    cs = slice(qc * 512, (qc + 1) * 512)
                    for h in range(2):
                        hp = slice(h * 64, (h + 1) * 64)
                        nc.tensor.matmul(
                            ps[h][:, cs],
                            lhsT=kt[hp, ms],
                            rhs=q2T[(b, qp)][hp, cs],
                            start=True,
                            stop=True,
                        )
                for h in range(2):
                    s = stp.tile([128, 1024], F16, tag="st", name=f"st{b}{m}{qp}{h}")
                    nc.scalar.activation(s[:], ps[h][:], AF.Exp, scale=SCALE)
                    st[(b, h, m, qp)] = s

            def attnv(b, qp, m):
                if m == 0:
                    ps_o[(b, qp)] = op.tile(
                        [128, 1024], F32, tag="o", name=f"o{b}{qp}"
                    )
                    if qp == 0:
                        ps_d[b] = dp.tile([128, 1024], F32, tag="d", name=f"d{b}")
                po = ps_o[(b, qp)]
                pd = ps_d[b]
                kw = dict(start=(m == 0), stop=(m == 15))
                for qc in range(2):
                    cs = slice(qc * 512, (qc + 1) * 512)
                    for h in range(2):
                        nc.tensor.matmul(
                            po[h * 64 : (h + 1) * 64, cs],
                            lhsT=vpos[b][:, h * 1024 + m * 64 : h * 1024 + (m + 1) * 64],
                            rhs=st[(b, h, m, qp)][:, cs],
                            **kw,
                        )
                ds = slice(qp * 512, (qp + 1) * 512)
                for qc in range(2):
                    cs = slice(qc * 512, (qc + 1) * 512)
                    for h in range(2):
                        row = h * 32 + qc * 64
                        nc.tensor.matmul(
                            pd[row : row + 1, ds],
                            lhsT=ones_sb[:, 0:1],
                            rhs=st[(b, h, m, qp)][:, cs],
                            skip_group_check=True,
                            tile_position=(0, row),
                            **kw,
                        )

            def normalize(b, qp):
                # ps_o eviction first (frees the o slot the next attnv pass
                # blocks on), then one approx-reciprocal over every denom
                # row at once (frees d), then broadcast + multiply from
                # SBUF off the critical path.
                po = ps_o[(b, qp)]
                pd = ps_d[b]
                ds = slice(qp * 512, (qp + 1) * 512)
                rc = rcpool.tile([128, 1024], F32, tag="rc", name=f"rc{b}{qp}")
                ev_o = pep.tile([128, 1024], F32, tag="pe", name=f"evo{b}{qp}")
                bcast = [0] * 32
                nc.vector.tensor_copy(ev_o[:], po[:])
                # denom rows (h,qc) -> h*32 + qc*64; ~18-bit reciprocal is
                # ample for a softmax denominator; garbage lanes are never
                # selected by the broadcast below.
                nc.vector.reciprocal_approx_fast(rc[:, 0:512], pd[:, ds])
                c0 = slice(0, 512)
                c1 = slice(512, 1024)
                # qc1 dests (cols 512+) first: sources rows 64/96 cols 0:512
                for dst, srow in ((0, 64), (32, 64), (64, 96), (96, 96)):
                    nc.vector.stream_shuffle(
                        rc[dst : dst + 32, c1], rc[srow : srow + 32, c0], bcast
                    )
                # then qc0 dests in overwrite-safe order
                for dst, srow in ((64, 32), (96, 32), (32, 0), (0, 0)):
                    nc.vector.stream_shuffle(
                        rc[dst : dst + 32, c0], rc[srow : srow + 32, c0], bcast
                    )
                nc.vector.tensor_mul(outT[(b, qp)][:], ev_o[:], rc[:])

            def proj_piece(b, qp, pm, use_act=False):
                # one 128-position tile of the output projection
                ps_p = scp.tile([128, 1024], F32, tag="sc", name=f"pp{b}{qp}{pm}")
                for ncol in range(2):
                    nc.tensor.matmul(
                        ps_p[:, ncol * 512 : (ncol + 1) * 512],
                        lhsT=outT[(b, qp)][:, pm * 128 : (pm + 1) * 128],
                        rhs=wp_sb[:, ncol * 512 : (ncol + 1) * 512],
                        start=True,
                        stop=True,
                    )
                ev = pep.tile([128, 1024], F32, tag="pe", name=f"pe{b}{qp}{pm}")
                if use_act:
                    # tail only: ScalarE is idle, split the eviction
                    nc.vector.tensor_copy(ev[:, 0:512], ps_p[:, 0:512])
                    nc.scalar.copy(ev[:, 512:1024], ps_p[:, 512:1024])
                else:
                    # mid-stream: keep ScalarE exclusively on exp
                    nc.vector.tensor_copy(ev[:], ps_p[:])
                rows = slice(qp * 1024 + pm * 128, qp * 1024 + (pm + 1) * 128)
                nc.sync.dma_start(outp[b, rows, :], ev[:])

            def drain(gen):
                for _ in gen:
                    pass

            # ---- S0: minimal prefix — what scores(b0, m<8, qp0) needs -----
            drain(q_quarter(0, 0, scp, "sc"))
            drain(kv_quarter(0, 0, scp, "sc", op, "o"))

            late_consts()

            # ---- U0a (m 0-7): scores(b0,qp0) | rest of b0 qkv -------------
            def u0a_gen():
                yield from q_quarter(0, 1, op, "o")
                yield from kv_quarter(0, 1, op, "o", dp, "d")

            gen = u0a_gen()
            for m in range(8):
                scores(0, m, 0)
                next(gen, None)
            drain(gen)

            # ---- U0b (m 8-15): scores(b0,qp0) | b1 qkv + both transposes --
            def u0b_gen():
                yield from transposes(0)
                yield from q_quarter(1, 0, op, "o")
                yield from q_quarter(1, 1, op, "o")
                yield from kv_quarter(1, 0, op, "o", dp, "d")
                yield from kv_quarter(1, 1, op, "o", dp, "d")
                yield from transposes(1)

            gen = u0b_gen()
            for m in range(8, 16):
                scores(0, m, 0)
                next(gen, None)
                next(gen, None)
                next(gen, None)
            drain(gen)

            # ---- U1: scores(b0,qp1) | attnv(b0,qp0) -----------------------
            for m in range(16):
                scores(0, m, 1)
                attnv(0, 0, m)
            normalize(0, 0)
            # ---- U2: scores(b1,qp0) | attnv(b0,qp1) | proj(b0,qp0) --------
            for m in range(16):
                scores(1, m, 0)
                attnv(0, 1, m)
                if m % 2 == 1:
                    proj_piece(0, 0, m // 2)
            normalize(0, 1)
            # ---- U3: scores(b1,qp1) | attnv(b1,qp0) | proj(b0,qp1) --------
            for m in range(16):
                scores(1, m, 1)
                attnv(1, 0, m)
                if m % 2 == 1:
                    proj_piece(0, 1, m // 2)
            normalize(1, 0)
            # ---- U4: attnv(b1,qp1) | proj(b1,qp0) -------------------------
            for m in range(16):
                attnv(1, 1, m)
                if m % 2 == 1:
                    proj_piece(1, 0, m // 2, use_act=True)
            normalize(1, 1)
            for pm in range(8):
                proj_piece(1, 1, pm, use_act=True)
    return nc


def _get_compiled():
    global _COMPILED_NC
    if _COMPILED_NC is None:
        nc = bacc.Bacc(
            "TRN2", target_bir_lowering=False, debug=False, num_devices=NCORES
        )
        _emit(nc)
        nc.compile()
        _COMPILED_NC = nc
    return _COMPILED_NC


def _install_trace_shim():
    """Register antenv.axon_hooks NTFF hook (missing on this image)."""
    import contextlib
    import ctypes
    import types

    if "antenv.axon_hooks" in sys.modules:
        return
    try:
        import antenv
    except ImportError:
        return
    so_path = "/opt/axon/libaxon_pjrt.so"
    if not os.path.exists(so_path):
        return

    mod = types.ModuleType("antenv.axon_hooks")
    mod._hook = None
    mod.set_axon_ntff_profile_hook = lambda h: setattr(mod, "_hook", h)
    mod.get_axon_ntff_profile_hook = lambda: mod._hook

    lib = ctypes.CDLL(so_path)
    if not hasattr(lib, "axon_start_nrt_profile"):
        return
    lib.axon_start_nrt_profile.argtypes = [
        ctypes.POINTER(ctypes.c_int64),
        ctypes.c_size_t,
    ]
    lib.axon_start_nrt_profile.restype = ctypes.c_int64
    lib.axon_stop_nrt_profile.argtypes = [ctypes.c_char_p]
    lib.axon_stop_nrt_profile.restype = ctypes.c_int64

    @contextlib.contextmanager
    def _hook(output_dir, device_ids):
        import jax

        jax.devices()
        if device_ids:
            ids = (ctypes.c_int64 * len(device_ids))(*device_ids)
            rc = lib.axon_start_nrt_profile(ids, len(device_ids))
        else:
            rc = lib.axon_start_nrt_profile(None, 0)
        if rc != 0:
            raise RuntimeError(f"axon_start_nrt_profile rc={rc}")
        try:
            yield
        finally:
            n = lib.axon_stop_nrt_profile(str(output_dir).encode())
            if n < 0:
                raise RuntimeError(f"axon_stop_nrt_profile rc={n}")

    mod.set_axon_ntff_profile_hook(_hook)
    sys.modules["antenv.axon_hooks"] = mod
    antenv.axon_hooks = mod


def kernel(query, target, Wq, Wk, Wv, Wp, bp):
    global LAST_EXEC_NS, LAST_RESULTS
    query = np.asarray(query, dtype=np.float32)
    target = np.asarray(target, dtype=np.float32)
    Wq = np.asarray(Wq, dtype=np.float32)
    Wk = np.asarray(Wk, dtype=np.float32)
    Wv = np.asarray(Wv, dtype=np.float32)
    Wp = np.asarray(Wp, dtype=np.float32)
    bp = np.asarray(bp, dtype=np.float32)

    xq = np.ascontiguousarray(query.reshape(B * N, C).T).astype(np.float16)
    xt = np.ascontiguousarray(target.reshape(B * N, C).T).astype(np.float16)
    id64 = np.zeros((128, 64), dtype=np.float16)
    for p in range(128):
        id64[p, p % 64] = 1.0

    def wlayout(Wm, rows):
        # SBUF weight tile [p, kc*128 + m] = W[row0 + m, kc*128 + p]
        ws = Wm[rows, :].astype(np.float16)  # (128, 1024)
        return np.ascontiguousarray(
            ws.reshape(128, 8, 128).transpose(2, 1, 0).reshape(128, 1024)
        )

    in_maps = []
    for c in range(NCORES):
        rows = slice(c * 128, (c + 1) * 128)
        in_maps.append(
            {
                "xq": xq,
                "xt": xt,
                "wq": wlayout(Wq, rows),
                "wk": wlayout(Wk, rows),
                "wv": wlayout(Wv, rows),
                "wp": np.ascontiguousarray(Wp[:, rows].T).astype(np.float16),
                "id64": id64,
            }
        )

    if TRACE:
        _install_trace_shim()

    nc = _get_compiled()
    res = run_bass_kernel_spmd(
        nc, in_maps, core_ids=list(range(NCORES)), trace=TRACE
    )
    LAST_RESULTS = res
    LAST_EXEC_NS = res.exec_time_ns

    acc = res.results[0]["out_p"].astype(np.float64)
    for c in range(1, NCORES):
        acc += res.results[c]["out_p"]
    out = acc.astype(np.float32) + bp[None, None, :]
    return out


# revision 27
# speedup vs baseline: 1.0626x; 1.0398x over previous
"""Trainium2 Bass kernel for nn_CrossAttention (B=2, N=2048, C=1024, H=16).

Sharding: 16 heads / 8 cores = 2 heads per core (both batches on every
core).  Each core computes its heads' Q/K/V projections with the matching
128-row slice of Wq/Wk/Wv, full attention for its 4 (batch, head) pairs,
and a partial output projection against its 128-column slice of Wp.  The
host sums the 8 partial projections (the tensor-parallel all-reduce) and
adds the bias.

Schedule (per core): the exp of 16.8M score elements on the Scalar engine
(~135us at [128,1024] grain) is the pacing resource.  Work is organized
as four scores units (batch x query-half) streaming through ScalarE,
with attnv/denominator matmuls for the previous unit, Q/K/V projection
quarters, PE v-transposes, and output-projection pieces woven into each
unit's PE slack.  All large SBUF tensors are split per (batch, half) so
Tile's per-tile dependency tracking doesn't serialize across units.

PSUM budget (8 banks of 2KB/partition):
  sc pool   2 x [128,1024] f32 = 4 banks  (score regions / S0 qkv / proj)
  o  pool   1 x [128,1024] f32 = 2 banks  (attnv accum / woven qkv)
  d  pool   1 x [128,1024] f32 = 2 banks  (denominators / v-transposes)

On-device layouts (per core, fp16 matmul operands / fp32 PSUM):
  xq/xt    [1024 ch, 4096 pos]  channel-major inputs (host pre-transposed)
  q2T/k2T/v2T  4 x [128, 1024]  per (batch, half); rows 0-63 head0 dims
  vpos     2 x [128, 2048]      per batch, v transposed key-major via PE
  S_T      [128 keys, 1024 q]   exp(scores^T) tile per (b, h, m, qp), fp16
  outT     4 x [128, 1024]      normalized attention output per (b, qp)
  out_p    [2, 2048, 1024] f32  partial projection (summed on host)
"""

import os
import sys

for _p in ("/opt/trn_rl_repo", os.path.expanduser("~/.axon_site/_ro/trn_rl_repo")):
    if os.path.isdir(_p) and _p not in sys.path:
        sys.path.insert(0, _p)

import numpy as np

import concourse.bacc as bacc
import concourse.mybir as mybir
import concourse.tile as tile
from concourse.bass_utils import run_bass_kernel_spmd

F16 = mybir.dt.float16
F32 = mybir.dt.float32
AF = mybir.ActivationFunctionType

B, N, C, H, D = 2, 2048, 1024, 16, 64
NCORES = 8
SCALE = float(D) ** -0.5

TRACE = False
LAST_EXEC_NS = None
LAST_RESULTS = None

_COMPILED_NC = None


def _emit(nc):
    xq = nc.dram_tensor("xq", [C, B * N], F16, kind="ExternalInput")
    xt = nc.dram_tensor("xt", [C, B * N], F16, kind="ExternalInput")
    wq = nc.dram_tensor("wq", [128, 1024], F16, kind="ExternalInput")
    wk = nc.dram_tensor("wk", [128, 1024], F16, kind="ExternalInput")
    wv = nc.dram_tensor("wv", [128, 1024], F16, kind="ExternalInput")
    wp = nc.dram_tensor("wp", [128, C], F16, kind="ExternalInput")
    id64 = nc.dram_tensor("id64", [128, 64], F16, kind="ExternalInput")
    outp = nc.dram_tensor("out_p", [B, N, C], F32, kind="ExternalOutput")

    with tile.TileContext(nc) as tc:
        with (
            tc.tile_pool(name="consts", bufs=1) as cpool,
            tc.tile_pool(name="xs", bufs=6) as xs,
            tc.tile_pool(name="big", bufs=1) as big,
            tc.tile_pool(name="stp", bufs=40) as stp,
            tc.tile_pool(name="ob", bufs=1) as obp,
            tc.tile_pool(name="rc", bufs=2) as rcpool,
            tc.tile_pool(name="pe", bufs=3) as pep,
            tc.tile_pool(name="sc", bufs=2, space="PSUM") as scp,
            tc.tile_pool(name="o", bufs=1, space="PSUM") as op,
            tc.tile_pool(name="d", bufs=1, space="PSUM") as dp,
        ):
            # ---- constants ------------------------------------------------
            w_sb = {}
            for name, dram in (("wq", wq), ("wk", wk), ("wv", wv)):
                t_ = cpool.tile([128, 1024], F16, tag=name, name=f"w_{name}")
                nc.sync.dma_start(t_[:], dram[:])
                w_sb[name] = t_
            wp_sb = cpool.tile([128, C], F16, tag="wp")
            id_sb = cpool.tile([128, 64], F16, tag="id64")
            ones_sb = cpool.tile([128, 1], F16, tag="ones")
            nc.vector.memset(ones_sb[:], 1.0)

            def late_consts():
                nc.sync.dma_start(id_sb[:], id64[:])
                nc.sync.dma_start(wp_sb[:], wp[:])

            # per-(batch, half) activation tiles, [128, 1024] fp16 each
            q2T = {}
            k2T = {}
            v2T = {}
            vpos = {}
            outT = {}
            for b in range(2):
                vpos[b] = big.tile([128, 2048], F16, tag=f"vpos{b}", name=f"vpos{b}")
                for hf in range(2):
                    q2T[(b, hf)] = big.tile([128, 1024], F16, tag=f"q{b}{hf}", name=f"q2T{b}{hf}")
                    k2T[(b, hf)] = big.tile([128, 1024], F16, tag=f"k{b}{hf}", name=f"k2T{b}{hf}")
                    v2T[(b, hf)] = big.tile([128, 1024], F16, tag=f"v{b}{hf}", name=f"v2T{b}{hf}")
                    outT[(b, hf)] = obp.tile(
                        [128, 1024], F16, tag=f"outT{b}{hf}", name=f"outT{b}{hf}"
                    )

            st = {}    # (b, h, m, qp) -> S_T tile
            ps_o = {}  # (b, qp) -> attnv accumulator
            ps_d = {}  # b -> denominator psum

            # ---- emission helpers -----------------------------------------
            def q_quarter(b, hf, pool, ptag):
                cols = slice(b * 2048 + hf * 1024, b * 2048 + (hf + 1) * 1024)
                ps_q = pool.tile([128, 1024], F32, tag=ptag, name=f"psq{b}{hf}")
                for kc in range(8):
                    x_t = xs.tile([128, 1024], F16, tag="x", name=f"xq{b}{hf}{kc}")
                    nc.sync.dma_start(x_t[:], xq[kc * 128 : (kc + 1) * 128, cols])
                    for qc in range(2):
                        cs = slice(qc * 512, (qc + 1) * 512)
                        nc.tensor.matmul(
                            ps_q[:, cs],
                            lhsT=w_sb["wq"][:, kc * 128 : (kc + 1) * 128],
                            rhs=x_t[:, cs],
                            start=(kc == 0),
                            stop=(kc == 7),
                        )
                    if kc % 2 == 1:
                        yield
                nc.vector.tensor_copy(q2T[(b, hf)][:], ps_q[:])

            def kv_quarter(b, hf, poolk, ktag, poolv, vtag):
                cols = slice(b * 2048 + hf * 1024, b * 2048 + (hf + 1) * 1024)
                ps_k = poolk.tile([128, 1024], F32, tag=ktag, name=f"psk{b}{hf}")
                ps_v = poolv.tile([128, 1024], F32, tag=vtag, name=f"psv{b}{hf}")
                for kc in range(8):
                    x_t = xs.tile([128, 1024], F16, tag="x", name=f"xt{b}{hf}{kc}")
                    nc.sync.dma_start(x_t[:], xt[kc * 128 : (kc + 1) * 128, cols])
                    for qc in range(2):
                        cs = slice(qc * 512, (qc + 1) * 512)
                        nc.tensor.matmul(
                            ps_k[:, cs],
                            lhsT=w_sb["wk"][:, kc * 128 : (kc + 1) * 128],
                            rhs=x_t[:, cs],
                            start=(kc == 0),
                            stop=(kc == 7),
                        )
                        nc.tensor.matmul(
                            ps_v[:, cs],
                            lhsT=w_sb["wv"][:, kc * 128 : (kc + 1) * 128],
                            rhs=x_t[:, cs],
                            start=(kc == 0),
                            stop=(kc == 7),
                        )
                    if kc % 2 == 1:
                        yield
                nc.vector.tensor_copy(k2T[(b, hf)][:], ps_k[:])
                nc.vector.tensor_copy(v2T[(b, hf)][:], ps_v[:])

            def transposes(b):
                for h in range(2):
                    hp = slice(h * 64, (h + 1) * 64)
                    for oct_ in range(2):
                        ps_t = dp.tile(
                            [128, 512], F16, tag="d", name=f"pst{b}{h}{oct_}"
                        )
                        for i in range(8):
                            m = oct_ * 8 + i
                            src = v2T[(b, m // 8)]
                            ks = slice((m % 8) * 128, (m % 8 + 1) * 128)
                            nc.tensor.transpose(
                                ps_t[:, i * 64 : (i + 1) * 64],
                                src[hp, ks],
                                id_sb[hp, :],
                            )
                        nc.vector.tensor_copy(
                            vpos[b][
                                :,
                                h * 1024 + oct_ * 512 : h * 1024 + (oct_ + 1) * 512,
                            ],
                            ps_t[:],
                        )
                        yield

            def scores(b, m, qp):
                ps = [
                    scp.tile([128, 1024], F32, tag="sc", name=f"sc{b}{m}{qp}{h}")
                    for h in range(2)
                ]
                kt = k2T[(b, m // 8)]
                ms = slice((m % 8) * 128, (m % 8 + 1) * 128)
                for qc in range(2):
                    cs = slice(qc * 512, (qc + 1) * 512)
                    for h in range(2):
                        hp = slice(h * 64, (h + 1) * 64)
                        nc.tensor.matmul(
                            ps[h][:, cs],
                            lhsT=kt[hp, ms],
                            rhs=q2T[(b, qp)][hp, cs],
                            start=True,
                            stop=True,
                        )
                for h in range(2):
                    s = stp.tile([128, 1024], F16, tag="st", name=f"st{b}{m}{qp}{h}")
                    nc.scalar.activation(s[:], ps[h][:], AF.Exp, scale=SCALE)
                    st[(b, h, m, qp)] = s

            def attnv(b, qp, m):
                if m == 0:
                    ps_o[(b, qp)] = op.tile(
                        [128, 1024], F32, tag="o", name=f"o{b}{qp}"
                    )
                    if qp == 0:
                        ps_d[b] = dp.tile([128, 1024], F32, tag="d", name=f"d{b}")
                po = ps_o[(b, qp)]
                pd = ps_d[b]
                kw = dict(start=(m == 0), stop=(m == 15))
                for qc in range(2):
                    cs = slice(qc * 512, (qc + 1) * 512)
                    for h in range(2):
                        nc.tensor.matmul(
                            po[h * 64 : (h + 1) * 64, cs],
                            lhsT=vpos[b][:, h * 1024 + m * 64 : h * 1024 + (m + 1) * 64],
                            rhs=st[(b, h, m, qp)][:, cs],
                            **kw,
                        )
                ds = slice(qp * 512, (qp + 1) * 512)
                for qc in range(2):
                    cs = slice(qc * 512, (qc + 1) * 512)
                    for h in range(2):
                        row = h * 32 + qc * 64
                        nc.tensor.matmul(
                            pd[row : row + 1, ds],
                            lhsT=ones_sb[:, 0:1],
                            rhs=st[(b, h, m, qp)][:, cs],
                            skip_group_check=True,
                            tile_position=(0, row),
                            **kw,
                        )

            def normalize(b, qp):
                # ps_o eviction first (frees the o slot the next attnv pass
                # blocks on), then one approx-reciprocal over every denom
                # row at once (frees d), then broadcast + multiply from
                # SBUF off the critical path.
                po = ps_o[(b, qp)]
                pd = ps_d[b]
                ds = slice(qp * 512, (qp + 1) * 512)
                rc = rcpool.tile([128, 1024], F32, tag="rc", name=f"rc{b}{qp}")
                ev_o = pep.tile([128, 1024], F32, tag="pe", name=f"evo{b}{qp}")
                bcast = [0] * 32
                nc.vector.tensor_copy(ev_o[:], po[:])
                # denom rows (h,qc) -> h*32 + qc*64; ~18-bit reciprocal is
                # ample for a softmax denominator; garbage lanes are never
                # selected by the broadcast below.
                nc.vector.reciprocal_approx_fast(rc[:, 0:512], pd[:, ds])
                c0 = slice(0, 512)
                c1 = slice(512, 1024)
                # qc1 dests (cols 512+) first: sources rows 64/96 cols 0:512
                for dst, srow in ((0, 64), (32, 64), (64, 96), (96, 96)):
                    nc.vector.stream_shuffle(
                        rc[dst : dst + 32, c1], rc[srow : srow + 32, c0], bcast
                    )
                # then qc0 dests in overwrite-safe order
                for dst, srow in ((64, 32), (96, 32), (32, 0), (0, 0)):
                    nc.vector.stream_shuffle(
                        rc[dst : dst + 32, c0], rc[srow : srow + 32, c0], bcast
                    )
                nc.vector.tensor_mul(outT[(b, qp)][:], ev_o[:], rc[:])

            def proj_piece(b, qp, pm, use_act=False):
                # one 128-position tile of the output projection
                ps_p = scp.tile([128, 1024], F32, tag="sc", name=f"pp{b}{qp}{pm}")
                for ncol in range(2):
                    nc.tensor.matmul(
                        ps_p[:, ncol * 512 : (ncol + 1) * 512],
                        lhsT=outT[(b, qp)][:, pm * 128 : (pm + 1) * 128],
                        rhs=wp_sb[:, ncol * 512 : (ncol + 1) * 512],
                        start=True,
                        stop=True,
                    )
                ev = pep.tile([128, 1024], F32, tag="pe", name=f"pe{b}{qp}{pm}")
                if use_act:
                    # tail only: ScalarE is idle, split the eviction
                    nc.vector.tensor_copy(ev[:, 0:512], ps_p[:, 0:512])
                    nc.scalar.copy(ev[:, 512:1024], ps_p[:, 512:1024])
                else:
                    # mid-stream: keep ScalarE exclusively on exp
                    nc.vector.tensor_copy(ev[:], ps_p[:])
                rows = slice(qp * 1024 + pm * 128, qp * 1024 + (pm + 1) * 128)
                nc.sync.dma_start(outp[b, rows, :], ev[:])

            def drain(gen):
                for _ in gen:
                    pass

            # ---- S0: minimal prefix — what scores(b0, m<8, qp0) needs -----
            drain(q_quarter(0, 0, scp, "sc"))
            drain(kv_quarter(0, 0, scp, "sc", op, "o"))

            late_consts()

            # ---- U0a (m 0-7): scores(b0,qp0) | rest of b0 qkv -------------
            def u0a_gen():
                yield from q_quarter(0, 1, op, "o")
                yield from kv_quarter(0, 1, op, "o", dp, "d")

            gen = u0a_gen()
            for m in range(8):
                scores(0, m, 0)
                next(gen, None)
            drain(gen)

            # ---- U0b (m 8-15): scores(b0,qp0) | b1 qkv + both transposes --
            def u0b_gen():
                yield from transposes(0)
                yield from q_quarter(1, 0, op, "o")
                yield from q_quarter(1, 1, op, "o")
                yield from kv_quarter(1, 0, op, "o", dp, "d")
                yield from kv_quarter(1, 1, op, "o", dp, "d")
                yield from transposes(1)

            gen = u0b_gen()
            for m in range(8, 16):
                scores(0, m, 0)
                next(gen, None)
                next(gen, None)
                next(gen, None)
            drain(gen)

            # ---- U1: scores(b0,qp1) | attnv(b0,qp0) -----------------------
            for m in range(16):
                scores(0, m, 1)
                attnv(0, 0, m)
            normalize(0, 0)
            # ---- U2: scores(b1,qp0) | attnv(b0,qp1) | proj(b0,qp0) --------
            for m in range(16):
                scores(1, m, 0)
                attnv(0, 1, m)
                if m % 2 == 1:
                    proj_piece(0, 0, m // 2)
            normalize(0, 1)
            # ---- U3: scores(b1,qp1) | attnv(b1,qp0) | proj(b0,qp1) --------
            for m in range(16):
                scores(1, m, 1)
                attnv(1, 0, m)
                if m % 2 == 1:
                    proj_piece(0, 1, m // 2)
            normalize(1, 0)
            # ---- U4: attnv(b1,qp1) | proj(b1,qp0) -------------------------
            for m in range(16):
                attnv(1, 1, m)
                if m % 2 == 1:
                    proj_piece(1, 0, m // 2, use_act=True)
            normalize(1, 1)
            for pm in range(8):
                proj_piece(1, 1, pm, use_act=True)
    return nc


def _get_compiled():
    global _COMPILED_NC
    if _COMPILED_NC is None:
        nc = bacc.Bacc(
            "TRN2", target_bir_lowering=False, debug=False, num_devices=NCORES
        )
        _emit(nc)
        nc.compile()
        _COMPILED_NC = nc
    return _COMPILED_NC


def _install_trace_shim():
    """Register antenv.axon_hooks NTFF hook (missing on this image)."""
    import contextlib
    import ctypes
    import types

    if "antenv.axon_hooks" in sys.modules:
        return
    try:
        import antenv
    except ImportError:
        return
    so_path = "/opt/axon/libaxon_pjrt.so"
    if not os.path.exists(so_path):
        return

    mod = types.ModuleType("antenv.axon_hooks")
    mod._hook = None
    mod.set_axon_ntff_profile_hook = lambda h: setattr(mod, "_hook", h)
    mod.get_axon_ntff_profile_hook = lambda: mod._hook

    lib = ctypes.CDLL(so_path)
    if not hasattr(lib, "axon_start_nrt_profile"):
        return
    lib.axon_start_nrt_profile.argtypes = [
        ctypes.POINTER(ctypes.c_int64),
        ctypes.c_size_t,
    ]
    lib.axon_start_nrt_profile.restype = ctypes.c_int64
    lib.axon_stop_nrt_profile.argtypes = [ctypes.c_char_p]
    lib.axon_stop_nrt_profile.restype = ctypes.c_int64

    @contextlib.contextmanager
    def _hook(output_dir, device_ids):
        import jax

        jax.devices()
        if device_ids:
            ids = (ctypes.c_int64 * len(device_ids))(*device_ids)
            rc = lib.axon_start_nrt_profile(ids, len(device_ids))
        else:
            rc = lib.axon_start_nrt_profile(None, 0)
        if rc != 0:
            raise RuntimeError(f"axon_start_nrt_profile rc={rc}")
        try:
            yield
        finally:
            n = lib.axon_stop_nrt_profile(str(output_dir).encode())
            if n < 0:
                raise RuntimeError(f"axon_stop_nrt_profile rc={n}")

    mod.set_axon_ntff_profile_hook(_hook)
    sys.modules["antenv.axon_hooks"] = mod
    antenv.axon_hooks = mod


def kernel(query, target, Wq, Wk, Wv, Wp, bp):
    global LAST_EXEC_NS, LAST_RESULTS
    query = np.asarray(query, dtype=np.float32)
    target = np.asarray(target, dtype=np.float32)
    Wq = np.asarray(Wq, dtype=np.float32)
    Wk = np.asarray(Wk, dtype=np.float32)
    Wv = np.asarray(Wv, dtype=np.float32)
    Wp = np.asarray(Wp, dtype=np.float32)
    bp = np.asarray(bp, dtype=np.float32)

    xq = np.ascontiguousarray(query.reshape(B * N, C).T).astype(np.float16)
    xt = np.ascontiguousarray(target.reshape(B * N, C).T).astype(np.float16)
    id64 = np.zeros((128, 64), dtype=np.float16)
    for p in range(128):
        id64[p, p % 64] = 1.0

    def wlayout(Wm, rows):
        # SBUF weight tile [p, kc*128 + m] = W[row0 + m, kc*128 + p]
        ws = Wm[rows, :].astype(np.float16)  # (128, 1024)
        return np.ascontiguousarray(
            ws.reshape(128, 8, 128).transpose(2, 1, 0).reshape(128, 1024)
        )

    in_maps = []
    for c in range(NCORES):
        rows = slice(c * 128, (c + 1) * 128)
        in_maps.append(
            {
                "xq": xq,
                "xt": xt,
                "wq": wlayout(Wq, rows),
                "wk": wlayout(Wk, rows),
                "wv": wlayout(Wv, rows),
                "wp": np.ascontiguousarray(Wp[:, rows].T).astype(np.float16),
                "id64": id64,
            }
        )

    if TRACE:
        _install_trace_shim()

    nc = _get_compiled()
    res = run_bass_kernel_spmd(
        nc, in_maps, core_ids=list(range(NCORES)), trace=TRACE
    )
    LAST_RESULTS = res
    LAST_EXEC_NS = res.exec_time_ns

    acc = res.results[0]["out_p"].astype(np.float64)
    for c in range(1, NCORES):
        acc += res.results[c]["out_p"]
    out = acc.astype(np.float32) + bp[None, None, :]
    return out
